# revision 1
# baseline (speedup 1.0000x reference)
"""Trainium2 Bass kernel for nn_EndtoEndIntervetionMap.

Computes, for B=4,194,304 rows split evenly over 8 NeuronCores:
    beta = sigmoid(relu(x @ W1 + b1) @ W2 + b2)          (tiny MLP, per row)
    14 explicit-Euler SIR steps on y=(S,I,R) with that beta.

Design (per core, RC rows; v4 — contiguous DMA, hexa packing, 1024-col
streams, fp32r):
  All HBM traffic is contiguous (4 KB runs for x, 1.5 KB runs for y):
    x is loaded per-ST as [128, 1024] with partition p holding 128
    consecutive rows; y likewise holds 128-row runs per (s, p).
  MLP in "hexa" transposed layout:
    - PE transposes turn each [128,128] x-block into xT: chunklet c
      (the c-th row of every partition's 128-row span within block b)
      sits at partitions [8c, 8c+8); 16 chunklets per block, 8 blocks.
    - mm1 contracts K=128 = 16 chunklets x 8 features with a
      block-diagonal stationary packing -> M=128 = 16 chunklets x 8
      hidden units, over the FULL 1024-col stream (8 blocks x 128 rows).
      The 64 hidden units are processed as EIGHT 8-unit slices.
    - relu+bias evacuates PSUM->SBUF (ACT, 1-in-4 on Pool).
    - mm2 accumulates the 8 hidden slices (start/stop chain) with
      [K=128 = 16x8, M=32] packings of W2, leaving beta_pre for 16
      chunklets at PSUM partitions 0..15 - a legal base-0 strip.
    - one tiny PE transpose per block ([16,128] -> [128,16]) writes beta
      as btr[p, a] directly (a = 16b + c), matching the natural
      row-major y layout; no DMA shuffles, no tile_position needed.
      These transposes are software-pipelined one ST late so the PE
      never waits on the bq PSUM->SBUF evacuation; btr borrows the low
      128 columns of the xtt PSUM tile (WAR-tracked) to fit 8 banks.
  mm1/mm2 run as float32r (1 cyc/row vs 4 for plain f32 at moving-dim
  >= 256); the verifier requires fp32r inputs to come from ROUNDING
  producers, so the weights are rounded once via engine copies and the
  xt/h evacuation copies emit fp32r directly.
  SIR runs partition-major on DVE only, in sign-tracked scaled
  coordinates, 4 plain tensor_tensor/tensor_scalar ops per Euler step
  (sharing the n*m product).  Measured-critical HW facts: GPSIMD ops and
  TensorScalarPtr (fused scalar_tensor_tensor / AP-scalar tensor_scalar)
  are an order of magnitude slower than modeled, so neither appears on
  the steady-state path.  SIR ops are emitted as closures interleaved
  into the next group's ST stream so the serial chain never blocks the
  DVE queue ahead of MLP evacuations.  y-load prefetches on the ACT
  queue at group start; y-store sits alone on the SP queue (its long
  wait for SIR(g) must not block anything else).  Group sizes taper
  (8,8,8,4,2,2 STs) so the final group's exposed SIR+store tail is
  short.
"""

import sys

import numpy as np

for _p in ("/opt/trn_rl_repo",):
    if _p not in sys.path:
        sys.path.insert(0, _p)

import concourse.bass as bass
import concourse.mybir as mybir
from concourse import bacc
from concourse.bass_utils import run_bass_kernel_spmd
from concourse.tile import TileContext

F32 = mybir.dt.float32
F32R = mybir.dt.float32r
AF = mybir.ActivationFunctionType
OP = mybir.AluOpType

N_CORES = 8
GAMMA = 0.1
STEPS = 2
WINDOW = 7
N_ITER = WINDOW * STEPS  # 14
DT = 1.0 / STEPS  # 0.5
CDEC = 1.0 - DT * GAMMA  # 0.95

_NC_CACHE = {}


def build_nc(RC: int, ST: int = 16384, hw_loop: int = 1,
             trace_sim: bool = False, use_f32r: bool = True,
             relu_mode: str = "act", group_sizes=None, ablate: str = "",
             sir_mode: str = "dve4", wide_relu: bool = False):
    """Build the single-core Bass program for RC rows.

    hw_loop > 1 wraps the whole pipeline in a hardware For_i loop that
    re-executes it hw_loop times (timing only: per-iteration device time
    is (T(hw_loop=R) - T(hw_loop=1)) / (R - 1); outputs are unchanged
    since every iteration recomputes the same values).
    """
    A = ST // 128  # rows per partition in x_nat (=chunklets per ST)
    NB = A // 16  # transpose blocks (128 cols of x_nat) per ST
    NST = RC // ST
    assert RC == NST * ST and A == 128 and NB == 8
    if group_sizes is None:
        # taper the tail so the last group's serial SIR + store is short
        group_sizes = []
        left = NST
        while left > 22:
            group_sizes.append(8)
            left -= 8
        for gsz in (8, 8, 4, 2):
            if left >= gsz + 2:
                group_sizes.append(gsz)
                left -= gsz
        group_sizes.append(left)
    assert sum(group_sizes) == NST and all(g >= 1 for g in group_sizes)
    WMAX = max(group_sizes) * A

    DTR = F32R if use_f32r else F32  # rounded dtype for matmul inputs

    nc = bacc.Bacc(None, target_bir_lowering=False)

    x_d = nc.declare_dram_parameter("x", [RC, 8], F32, isOutput=False)
    y_d = nc.declare_dram_parameter("y", [RC, 3], F32, isOutput=False)
    w1e_d = nc.declare_dram_parameter("w1e", [128, 1024], F32, isOutput=False)
    w2e_d = nc.declare_dram_parameter("w2e", [128, 256], F32, isOutput=False)
    b1e_d = nc.declare_dram_parameter("b1e", [128, 8], F32, isOutput=False)
    b2b_d = nc.declare_dram_parameter("b2b", [128, 1], F32, isOutput=False)
    id_d = nc.declare_dram_parameter("ident", [128, 128], F32, isOutput=False)
    yo_d = nc.declare_dram_parameter("yout", [RC, 3], F32, isOutput=True)

    XCOLS = 8 * A  # 1024 x columns per partition

    with TileContext(nc, trace_sim=trace_sim) as tc:
        with (
            tc.tile_pool(name="consts", bufs=1) as cpool,
            tc.tile_pool(name="xnat", bufs=3) as xpool,
            tc.tile_pool(name="xtsb", bufs=2) as xtpool,
            tc.tile_pool(name="htsb", bufs=3) as hspool,
            tc.tile_pool(name="bqsb", bufs=2) as bqpool,
            tc.tile_pool(name="bpm", bufs=2) as bpool,
            tc.tile_pool(name="sir", bufs=1) as spool,
            tc.tile_pool(name="ynat", bufs=2) as ypool,
            tc.tile_pool(name="ps_xtt", bufs=1, space="PSUM") as ptpool,
            tc.tile_pool(name="ps_ht", bufs=(2 if wide_relu else 4), space="PSUM") as phpool,
            tc.tile_pool(name="ps_bq", bufs=1, space="PSUM") as pbpool,
        ):
            w1e = cpool.tile([128, 1024], F32)
            w2e = cpool.tile([128, 256], F32)
            b1e = cpool.tile([128, 8], F32)
            b2b = cpool.tile([128, 1], F32)
            ident = cpool.tile([128, 128], F32)
            nc.sync.dma_start(w1e[:], w1e_d[:])
            nc.sync.dma_start(w2e[:], w2e_d[:])
            nc.sync.dma_start(b1e[:], b1e_d[:])
            nc.sync.dma_start(b2b[:], b2b_d[:])
            nc.sync.dma_start(ident[:], id_d[:])
            # fp32r matmul inputs must be produced by rounding instructions
            # (a plain f32 DMA is rejected by the BIR verifier), so round
            # the weights once via engine copies
            w1r = cpool.tile([128, 1024], DTR)
            w2r = cpool.tile([128, 256], DTR)
            nc.vector.tensor_copy(w1r[:], w1e[:])
            nc.vector.tensor_copy(w2r[:], w2e[:])

            def relu_evac(idx, dst, src, bias):
                """relu+bias PSUM->SBUF (ACT/DVE only: Pool can't reach
                PSUM; Pool instead owns the SBUF-only SIR phase)."""
                if relu_mode == "act":
                    on_dve = False
                else:
                    on_dve = idx % 3 == 2  # ~5 of 16 per ST
                if on_dve:
                    nc.vector.tensor_scalar(dst, src, bias, 0.0, OP.add, OP.max)
                else:
                    nc.scalar.activation(dst, src, AF.Relu, bias=bias, scale=1.0)

            def emit_body():
                # deferred beta-transpose closures, one ST late (pipelined
                # past the bq PSUM->SBUF evacuation)
                pending_btr = []
                # deferred SIR closures of the previous group, interleaved
                # into this group's ST stream so the serial SIR chain never
                # blocks DVE/Pool queues ahead of MLP work
                pending_aux = []

                def emit_btr():
                    for fn in pending_btr:
                        fn()
                    pending_btr.clear()

                def pop_aux(n):
                    for fn in pending_aux[:n]:
                        fn()
                    del pending_aux[:n]

                gbase = 0
                for g, GRP in enumerate(group_sizes):
                    W = GRP * A
                    GR = GRP * ST  # rows in group
                    b_pm = bpool.tile([128, W], F32)
                    if "nomlp" in ablate:
                        nc.vector.memset(b_pm[:], 0.0)

                    # prefetch y for this group (ACT queue: its waits are
                    # pre-satisfied; SP holds only the stores, whose long
                    # SIR waits must not block loads)
                    y_nat = ypool.tile([128, 3 * W], F32)
                    if "noy" not in ablate:
                        nc.scalar.dma_start(
                            y_nat[:],
                            y_d[gbase : gbase + GR, :].rearrange(
                                "(s p t) c -> p s t c", s=GRP, p=128
                            ),
                        )

                    for s in range(GRP):
                        stbase = gbase + s * ST

                        # ---- x load: contiguous, partition p = 128 rows
                        x_nat = xpool.tile([128, XCOLS], F32)
                        nc.scalar.dma_start(
                            x_nat[:],
                            x_d[stbase : stbase + ST, :].rearrange(
                                "(p a) k -> p (a k)", p=128
                            ),
                        )

                        if "nomlp" in ablate:
                            continue
                        # ---- PE transposes: features onto partitions
                        xtt = ptpool.tile([128, 1024], F32, tag="xtt")
                        xt_sb = xtpool.tile([128, 1024], DTR)
                        for b in range(NB):
                            nc.tensor.transpose(
                                xtt[:, 128 * b : 128 * b + 128],
                                x_nat[:, 128 * b : 128 * b + 128],
                                ident[:],
                            )
                        nc.vector.tensor_copy(xt_sb[:], xtt[:])

                        # mm streams are 512 cols (a matmul out may not
                        # cross a PSUM bank); two half-chains (h = cb 0:4 /
                        # 4:8) accumulate into the two banks of bq
                        bq = pbpool.tile([128, 1024], F32, tag="bq")
                        for i in range(8):
                            if wide_relu:
                                # one 2-bank hT tile per slice; each matmul
                                # OUT stays within a bank, ONE wide relu
                                # drains both halves (halves ACT op count)
                                hTw = phpool.tile([128, 1024], F32, name="hTw")
                                hTw_sb = hspool.tile([128, 1024], DTR, name="hTw_sb")
                                for h in range(2):
                                    nc.tensor.matmul(
                                        hTw[:, 512 * h : 512 * h + 512],
                                        w1r[:, 128 * i : 128 * i + 128],
                                        xt_sb[:, 512 * h : 512 * h + 512],
                                    )
                                relu_evac(i, hTw_sb[:], hTw[:], b1e[:, i : i + 1])
                                for h in range(2):
                                    nc.tensor.matmul(
                                        bq[0:32, 512 * h : 512 * h + 512],
                                        w2r[:, 32 * i : 32 * i + 32],
                                        hTw_sb[:, 512 * h : 512 * h + 512],
                                        start=(i == 0),
                                        stop=(i == 7),
                                    )
                                continue
                            for h in range(2):
                                hT = phpool.tile([128, 512], F32)
                                hT_sb = hspool.tile([128, 512], DTR)
                                nc.tensor.matmul(
                                    hT[:],
                                    w1r[:, 128 * i : 128 * i + 128],
                                    xt_sb[:, 512 * h : 512 * h + 512],
                                )
                                relu_evac(
                                    2 * i + h, hT_sb[:], hT[:], b1e[:, i : i + 1]
                                )
                                nc.tensor.matmul(
                                    bq[0:32, 512 * h : 512 * h + 512],
                                    w2r[:, 32 * i : 32 * i + 32],
                                    hT_sb[:],
                                    start=(i == 0),
                                    stop=(i == 7),
                                )
                        bq_sb = bqpool.tile([16, 1024], F32)
                        nc.vector.tensor_copy(bq_sb[:], bq[0:16, :])

                        # beta transposes of the PREVIOUS ST run here (their
                        # bq_sb is long since evacuated -> no PE wait)
                        emit_btr()

                        def make_btr(bq_sb=bq_sb, xtt=xtt, b_pm=b_pm, s=s):
                            def fn():
                                # btr[p, a] with a = 16b + c, borrowed from
                                # the xtt tile's low 128 columns
                                btr = xtt[:, 0:128]
                                for b in range(NB):
                                    nc.tensor.transpose(
                                        btr[:, 16 * b : 16 * b + 16],
                                        bq_sb[:, 128 * b : 128 * b + 128],
                                        ident[0:16, 0:16],
                                    )
                                nc.vector.tensor_copy(
                                    b_pm[:, A * s : A * s + A], btr
                                )

                            return fn

                        pending_btr.append(make_btr())
                        pop_aux(5)

                    # flush the last ST's beta transposes before sigmoid
                    emit_btr()
                    # finish the previous group's SIR before queueing ours
                    pop_aux(len(pending_aux))

                    # ---- SIR phase for this group ----
                    # m-chain (2 tensor_tensor ops) on Pool, n-chain (one
                    # fused scalar_tensor_tensor) on DVE; Pool cannot fuse
                    # and cannot touch PSUM, so it owns the SBUF-only SIR
                    # work while ACT/DVE evacuate PSUM for the MLP.
                    t_pm = spool.tile([128, WMAX], F32, tag="t_pm", name="t_pm")[:, 0:W]
                    nc.scalar.activation(
                        t_pm, b_pm[:], AF.Sigmoid, bias=b2b[:, 0:1]
                    )
                    th = spool.tile([128, WMAX], F32, tag="th", name="th")[:, 0:W]
                    r2 = spool.tile([128, WMAX], F32, tag="r2", name="r2")[:, 0:W]
                    mA = spool.tile([128, WMAX], F32, tag="u", name="u")[:, 0:W]
                    nA = spool.tile([128, WMAX], F32, tag="v", name="v")[:, 0:W]
                    mB = spool.tile([128, WMAX], F32, tag="u2", name="u2")[:, 0:W]
                    nB = spool.tile([128, WMAX], F32, tag="v2", name="v2")[:, 0:W]
                    nm = spool.tile([128, WMAX], F32, tag="nm", name="nm")[:, 0:W]
                    pte = spool.tile([128, WMAX], F32, tag="pte", name="pte")[:, 0:W]

                    y3 = y_nat[:].rearrange("p (t c) -> p t c", c=3)
                    Sap = y3[:, :, 0]
                    Iap = y3[:, :, 1]
                    Rap = y3[:, :, 2]

                    def sir_closures(th=th, r2=r2, mA=mA, nA=nA, mB=mB, nB=nB,
                                     nm=nm, pte=pte, t_pm=t_pm, Sap=Sap,
                                     Iap=Iap, Rap=Rap, y_nat=y_nat,
                                     gbase=gbase, GR=GR, GRP=GRP):
                        ops = []
                        ops.append(lambda: nc.vector.tensor_scalar_mul(
                            th, t_pm, DT))
                        ops.append(lambda: nc.vector.reciprocal(r2, th))
                        ops.append(lambda: nc.vector.tensor_mul(mA, th, Sap))
                        ops.append(lambda: nc.vector.tensor_mul(nA, th, Iap))
                        # SIR steps in sign-tracked coordinates: after two
                        # transitional steps, (m, n) = (u, -v) is a fixed
                        # point of m' = (n+1)*m ; n' = (m+c)*n; each step is
                        # two fused scalar_tensor_tensor ops on DVE
                        cm, cn, am, an = mA, nA, mB, nB
                        for k in range(N_ITER):
                            op_m = OP.subtract if k < 2 else OP.add
                            op_n = OP.subtract if k == 1 else OP.add

                            def pool_m(cm=cm, cn=cn, am=am, op_m=op_m):
                                # m' = (n op_m 1)*m = n*m op_m m
                                nc.gpsimd.tensor_mul(nm, cn, cm)
                                nc.gpsimd.tensor_tensor(am, nm, cm, op_m)

                            def dve_n(cm=cm, cn=cn, an=an, op_n=op_n):
                                nc.vector.scalar_tensor_tensor(
                                    an, cm, CDEC, cn, op_n, OP.mult)

                            def dve_m4(cm=cm, cn=cn, am=am, op_m=op_m):
                                # no-Ptr form: nm once, then plain TT/TS ops
                                nc.vector.tensor_mul(nm, cn, cm)
                                nc.vector.tensor_tensor(am, nm, cm, op_m)

                            def dve_n4(cm=cm, cn=cn, an=an, op_n=op_n):
                                nc.vector.tensor_scalar_mul(pte, cn, CDEC)
                                nc.vector.tensor_tensor(an, nm, pte, op_n)

                            if sir_mode == "dve4":
                                ops.append(dve_m4)
                                ops.append(dve_n4)
                            else:
                                ops.append(pool_m)
                                ops.append(dve_n)
                            cm, cn, am, an = am, an, cm, cn

                        def finals(cm=cm, cn=cn):
                            nc.vector.tensor_mul(Sap, cm, r2)
                            nc.vector.scalar_tensor_tensor(
                                Iap, cn, -1.0, r2, OP.mult, OP.mult)
                            nc.vector.tensor_add(pte, Sap, Iap)
                            nc.vector.tensor_scalar(
                                Rap, pte, -1.0, 1.0, OP.mult, OP.add)

                        ops.append(finals)
                        ops.append(lambda: nc.sync.dma_start(
                            yo_d[gbase : gbase + GR, :].rearrange(
                                "(s p t) c -> p s t c", s=GRP, p=128
                            ),
                            y_nat[:],
                        ))
                        return ops

                    if "nosir" not in ablate:
                        pending_aux.extend(sir_closures())
                    gbase += GR

                # drain the final group's SIR + store
                pop_aux(len(pending_aux))

            if hw_loop > 1:
                with tc.For_i(0, hw_loop):
                    emit_body()
            else:
                emit_body()

    nc.compile()
    return nc


def _prep_consts(W1, b1, W2, b2):
    w1e = np.zeros((128, 1024), np.float32)
    w2e = np.zeros((128, 256), np.float32)
    b1e = np.zeros((128, 8), np.float32)
    for i in range(8):
        for c in range(16):
            w1e[8 * c : 8 * c + 8, 128 * i + 8 * c : 128 * i + 8 * c + 8] = W1[
                :, 8 * i : 8 * i + 8
            ]
            w2e[8 * c : 8 * c + 8, 32 * i + c] = W2[8 * i : 8 * i + 8, 0]
            b1e[8 * c : 8 * c + 8, i] = b1[8 * i : 8 * i + 8]
    b2b = np.full((128, 1), b2[0], np.float32)
    ident = np.eye(128, dtype=np.float32)
    return w1e, w2e, b1e, b2b, ident


def run_sharded(y, x, W1, b1, W2, b2, trace=False, hw_loop=1, **spmd_kwargs):
    """Shard over 8 cores, run, gather. Returns (out, BassKernelResults)."""
    y = np.ascontiguousarray(np.asarray(y, np.float32))
    x = np.ascontiguousarray(np.asarray(x, np.float32))
    W1 = np.asarray(W1, np.float32)
    b1 = np.asarray(b1, np.float32)
    W2 = np.asarray(W2, np.float32)
    b2 = np.asarray(b2, np.float32)

    B = y.shape[0]
    RC = B // N_CORES
    key = (RC, hw_loop)
    if key not in _NC_CACHE:
        _NC_CACHE[key] = build_nc(RC, hw_loop=hw_loop)
    nc = _NC_CACHE[key]

    w1e, w2e, b1e, b2b, ident = _prep_consts(W1, b1, W2, b2)
    in_maps = []
    for c in range(N_CORES):
        in_maps.append(
            {
                "x": x[c * RC : (c + 1) * RC],
                "y": y[c * RC : (c + 1) * RC],
                "w1e": w1e,
                "w2e": w2e,
                "b1e": b1e,
                "b2b": b2b,
                "ident": ident,
            }
        )
    res = run_bass_kernel_spmd(
        nc, in_maps, core_ids=list(range(N_CORES)), trace=trace, **spmd_kwargs
    )
    out = np.concatenate([res.results[c]["yout"] for c in range(N_CORES)], axis=0)
    return out, res


def kernel(y, x, W1, b1, W2, b2):
    out, _ = run_sharded(y, x, W1, b1, W2, b2)
    return out


if __name__ == "__main__":
    rng = np.random.default_rng(0)
    B = N_CORES * 16384 * 8
    y0 = rng.random((B, 3), np.float32)
    y0 /= y0.sum(axis=1, keepdims=True)
    x = rng.random((B, 8), np.float32)
    W1 = (rng.standard_normal((8, 64)) * 0.3).astype(np.float32)
    b1 = np.zeros(64, np.float32)
    W2 = (rng.standard_normal((64, 1)) * 0.3).astype(np.float32)
    b2 = np.zeros(1, np.float32)
    out = kernel(y0, x, W1, b1, W2, b2)
    print(out[:4], out.shape)

